# revision 3
# baseline (speedup 1.0000x reference)
"""Continuous positional bias kernel for Trainium2 (8 NeuronCores), v2.

Reference computation (per batch b):
    rel[q,k,:] = query_coords[b,q,:] - key_coords[b,k,:]
    h1 = relu(rel @ w1 + b1)      # (Nq,Nk,128)
    h2 = relu(h1 @ w2 + b2)       # (Nq,Nk,128)
    out[b,:,q,k] = (h2 @ w3 + b3).T

Layer 1 is linear in rel = q - k:
    w1^T rel + b1 = (w1^T q + b1) + (-w1^T k) = beta[:,q] + gamma[:,k]
gamma/beta computed on host; gamma held in SBUF as bf16 so the per-query
h1 = relu(gamma + beta_q) tensor_scalar runs in the packed DVE mode
(measured 481ns vs 748ns with fp32 src).

Per-query work and engine assignment (measured ns):
    h1   = relu(gamma + beta_q)     DVE tensor_scalar            481
    p2   = w2^T h1                  PE, 2 matmuls N=512          2x216
    h2   = relu(p2)                 ACT 7/8 queries (1114), DVE 1/8 (1224)
                                    b2 == 0 by spec -> no bias operand
    p3   = w3p^T h2                 PE, 2 col-tiled matmuls      2x216
    stage <- p3                     DVE copy [128,512]/2q        681
With the 7:1 relu split and copies on DVE, ACT and DVE both carry
~975ns/query; PE carries ~864ns/query plus hidden LDWEIGHTS.

The emission order software-pipelines with a 2-query shift so the PE
never waits on relu2: per step s it issues w2(s+2) then w3(s), while
relu2(s+1) runs on ACT/DVE.  PSUM: 3 x p2 [128,1024] (2 banks each) +
2 x p3 [128,512] = 8 banks.

Output is staged and DMA'd as bf16 (halves DMA bytes); host upcasts.
Sharding: 8 cores x (batch, 256 queries). Weights replicated.
"""

import numpy as np

B, NQ, NK, H, HD = 2, 1024, 1024, 8, 128
NCORES = 8
CPB = NCORES // B          # cores per batch = 4
QPC = NQ // CPB            # queries per core = 256
KT = 512                   # matmul moving free dim (one PSUM bank fp32)
NGR = 8                    # 2-query groups per staging round
RQ = 2 * NGR               # queries per staging round = 16
W3P = 32                   # w3 padded to 32 output columns (col-group width)
# relu2 engine split: queries with q % 32 in this set run on DVE, rest ACT
DVE_RELU_SET = frozenset((2, 7, 13, 18, 24, 29))
SHIFT = 2                  # w2 matmuls run this many queries ahead of relu2
W3LAG = 2                  # w3 matmuls trail relu2 by this many queries

_CACHE = {}


def _build_nc(with_bias):
    from contextlib import ExitStack

    import concourse.bass as bass
    import concourse.tile as tile
    from concourse import bacc, mybir
    from concourse.alu_op_type import AluOpType

    f32 = mybir.dt.float32
    bf16 = mybir.dt.bfloat16
    Relu = mybir.ActivationFunctionType.Relu

    nc = bacc.Bacc(
        "TRN2",
        target_bir_lowering=False,
        debug=False,
        enable_asserts=True,
        num_devices=NCORES,
    )

    gamma_d = nc.dram_tensor("gamma", (HD, NK), bf16, kind="ExternalInput").ap()
    beta_d = nc.dram_tensor("beta", (HD, QPC), f32, kind="ExternalInput").ap()
    w2_d = nc.dram_tensor("w2", (HD, HD), f32, kind="ExternalInput").ap()
    w3p_d = nc.dram_tensor("w3p", (HD, W3P), f32, kind="ExternalInput").ap()
    b2_d = nc.dram_tensor("b2", (HD, 1), f32, kind="ExternalInput").ap()
    out_d = nc.dram_tensor("out", (H, QPC, NK), bf16, kind="ExternalOutput").ap()

    with tile.TileContext(nc) as tc:
        with ExitStack() as ctx:
            consts = ctx.enter_context(tc.tile_pool(name="consts", bufs=1))
            h1p = ctx.enter_context(tc.tile_pool(name="h1p", bufs=6))
            h2p = ctx.enter_context(tc.tile_pool(name="h2p", bufs=6))
            stagep = ctx.enter_context(tc.tile_pool(name="stagep", bufs=2))
            ps2 = ctx.enter_context(tc.tile_pool(name="ps2", bufs=3, space="PSUM"))
            ps3 = ctx.enter_context(tc.tile_pool(name="ps3", bufs=2, space="PSUM"))

            # gamma feeds the first op of the pipeline: give it the sync
            # HWDGE queue alone; everything else loads in parallel on the
            # gpsimd SWDGE queue.
            gamma = consts.tile([HD, NK], bf16)
            nc.sync.dma_start(gamma, gamma_d)
            w2 = consts.tile([HD, HD], f32)
            nc.sync.dma_start(w2, w2_d)
            w3p = consts.tile([HD, W3P], f32)
            nc.sync.dma_start(w3p, w3p_d)
            b2 = consts.tile([HD, 1], f32)
            nc.sync.dma_start(b2, b2_d)
            beta = consts.tile([HD, QPC], f32)
            nc.sync.dma_start(beta, beta_d)

            w2r = consts.tile([HD, HD], bf16)
            nc.vector.tensor_copy(w2r, w2)
            w3pr = consts.tile([HD, W3P], bf16)
            nc.vector.tensor_copy(w3pr, w3p)

            h1t = [None] * QPC   # h1 tiles in flight
            h2t = [None] * QPC   # h2 tiles in flight
            p2t = [None] * QPC   # p2 psum tiles in flight
            p3t = {}             # group -> p3 psum tile
            staget = {}          # round -> stage tile

            def emit_h1(q):
                h1 = h1p.tile([HD, NK], bf16, tag="h1", name="h1")
                nc.vector.tensor_scalar(
                    h1, gamma, beta[:, q:q + 1], 0.0,
                    AluOpType.add, AluOpType.max,
                )
                h1t[q] = h1

            def emit_w2(q):
                p2 = ps2.tile([HD, NK], f32, tag="p2", name="p2")
                for kh in range(2):
                    nc.tensor.matmul(
                        p2[:, kh * KT:(kh + 1) * KT],
                        w2r,
                        h1t[q][:, kh * KT:(kh + 1) * KT],
                        start=True,
                        stop=True,
                    )
                p2t[q] = p2
                h1t[q] = None

            def emit_relu2(q):
                h2 = h2p.tile([HD, NK], bf16, tag="h2", name="h2")
                if with_bias:
                    nc.scalar.activation(h2, p2t[q], Relu, bias=b2)
                elif q % 32 in DVE_RELU_SET:
                    nc.vector.tensor_scalar(
                        h2, p2t[q], 0.0, None, AluOpType.max,
                    )
                else:
                    nc.scalar.activation(h2, p2t[q], Relu)
                h2t[q] = h2
                p2t[q] = None

            def emit_w3(q):
                g = q // 2
                if q % 2 == 0:
                    p3t[g] = ps3.tile([128, KT], f32, tag="p3", name="p3")
                for kh in range(2):
                    j = 2 * (q % 2) + kh
                    nc.tensor.matmul(
                        p3t[g][32 * j:32 * (j + 1), :],
                        w3pr,
                        h2t[q][:, kh * KT:(kh + 1) * KT],
                        start=True,
                        stop=True,
                        tile_position=(0, 32 * j),
                    )
                h2t[q] = None

            last_round = QPC // RQ - 1

            def emit_copy(g):
                # drain group g (queries 2g, 2g+1) into its round's stage.
                # In the last round ACT has run out of relu work, so split
                # the drains between ACT and DVE to shorten the tail.
                r, gi = divmod(g, NGR)
                if gi == 0:
                    staget[r] = stagep.tile([128, NGR * KT], bf16, tag="stage", name="stage")
                dst = staget[r][:, gi * KT:(gi + 1) * KT]
                if r == last_round and gi % 2 == 0:
                    nc.scalar.copy(dst, p3t[g])
                else:
                    nc.vector.tensor_copy(dst, p3t[g])
                del p3t[g]

            def emit_dma(r, c0, cn, final):
                # Staging layout: slot gi holds the 4-tile group for queries
                # (q0+2gi, q0+2gi+1); partition block 32j+0..8 holds heads for
                # (q offset j//2, k half j%2).  j 0/1 go out on the sync HWDGE
                # queue, j 2/3 on the gpsimd SWDGE queue (parallel channel,
                # gpsimd is otherwise idle).
                q0 = r * RQ
                for j in range(4):
                    dest = bass.AP(
                        tensor=out_d.tensor,
                        offset=out_d.offset + (q0 + 2 * c0 + (j // 2)) * NK
                        + (j % 2) * KT,
                        ap=[[QPC * NK, H], [2 * NK, cn], [1, KT]],
                    )
                    nc.sync.dma_start(
                        dest,
                        staget[r][32 * j:32 * j + H, c0 * KT:(c0 + cn) * KT],
                    )
                if final:
                    del staget[r]

            def after_copy(g):
                # fire DMAs once their slots are staged; the last round goes
                # out in two half-round chunks so the tail transfer is small
                r, gi = divmod(g, NGR)
                if gi == NGR - 1:
                    if r == last_round:
                        emit_dma(r, NGR // 2, NGR // 2, final=True)
                    else:
                        emit_dma(r, 0, NGR, final=True)
                elif r == last_round and gi == NGR // 2 - 1:
                    emit_dma(r, 0, NGR // 2, final=False)

            # --- software-pipelined emission ---
            for q in range(min(SHIFT + 1, QPC)):
                emit_h1(q)
            for q in range(min(SHIFT, QPC)):
                emit_w2(q)

            for s in range(QPC + W3LAG + 1):
                if s < QPC:
                    emit_relu2(s)
                if s + SHIFT + 1 < QPC:
                    emit_h1(s + SHIFT + 1)
                if s + SHIFT < QPC:
                    emit_w2(s + SHIFT)
                w3q = s - W3LAG
                if 0 <= w3q < QPC:
                    emit_w3(w3q)
                # drain a group one step after its second query's w3
                cpq = w3q - 1
                if cpq >= 1 and cpq % 2 == 1 and (cpq // 2) in p3t:
                    g = cpq // 2
                    emit_copy(g)
                    after_copy(g)
            # flush remaining groups
            for g in sorted(p3t.keys()):
                emit_copy(g)
                after_copy(g)

    nc.compile()
    return nc


def _get_nc(with_bias=False):
    key = ("nc", bool(with_bias))
    if key not in _CACHE:
        _CACHE[key] = _build_nc(with_bias)
    return _CACHE[key]


def make_in_maps(query_coords, key_coords, w1, b1, w2, b2, w3):
    """Host-side shard prep: per-core gamma/beta + replicated weights."""
    qc = np.asarray(query_coords, np.float32)
    kc = np.asarray(key_coords, np.float32)
    w1 = np.asarray(w1, np.float32)
    b1 = np.asarray(b1, np.float32)
    w2 = np.asarray(w2, np.float32)
    b2 = np.asarray(b2, np.float32)
    w3 = np.asarray(w3, np.float32)

    w3p = np.zeros((HD, W3P), np.float32)
    w3p[:, :H] = w3
    b2c = np.ascontiguousarray(b2.reshape(HD, 1))
    w2c = np.ascontiguousarray(w2)

    import ml_dtypes

    in_maps = []
    for c in range(NCORES):
        b = c // CPB
        q0 = (c % CPB) * QPC
        gamma = np.ascontiguousarray(
            (-(kc[b] @ w1).T).astype(ml_dtypes.bfloat16)         # (128, NK)
        )
        beta = np.ascontiguousarray(
            (qc[b, q0:q0 + QPC] @ w1).T + b1[:, None]            # (128, QPC)
        )
        in_maps.append(
            {"gamma": gamma, "beta": beta, "w2": w2c, "w3p": w3p, "b2": b2c}
        )
    return in_maps


def assemble_output(results, b3):
    """Gather per-core [H, QPC, NK] bf16 results into (B, H, NQ, NK) f32."""
    b3 = np.asarray(b3, np.float32)
    out = np.empty((B, H, NQ, NK), np.float32)
    for c in range(NCORES):
        b = c // CPB
        q0 = (c % CPB) * QPC
        out[b, :, q0:q0 + QPC, :] = np.asarray(results[c]["out"],
                                               dtype=np.float32)
    if np.any(b3):
        out += b3.reshape(1, H, 1, 1)
    return out


def kernel(**inputs):
    from concourse.bass_utils import run_bass_kernel_spmd

    in_maps = make_in_maps(
        inputs["query_coords"],
        inputs["key_coords"],
        inputs["w1"],
        inputs["b1"],
        inputs["w2"],
        inputs["b2"],
        inputs["w3"],
    )
    with_bias = bool(np.any(np.asarray(inputs["b2"], np.float32)))
    nc = _get_nc(with_bias)
    res = run_bass_kernel_spmd(nc, in_maps, list(range(NCORES)))
    return assemble_output(res.results, inputs["b3"])


# revision 4
# speedup vs baseline: 1.0198x; 1.0198x over previous
"""Continuous positional bias kernel for Trainium2 (8 NeuronCores), v2.

Reference computation (per batch b):
    rel[q,k,:] = query_coords[b,q,:] - key_coords[b,k,:]
    h1 = relu(rel @ w1 + b1)      # (Nq,Nk,128)
    h2 = relu(h1 @ w2 + b2)       # (Nq,Nk,128)
    out[b,:,q,k] = (h2 @ w3 + b3).T

Layer 1 is linear in rel = q - k:
    w1^T rel + b1 = (w1^T q + b1) + (-w1^T k) = beta[:,q] + gamma[:,k]
gamma/beta computed on host; gamma held in SBUF as bf16 so the per-query
h1 = relu(gamma + beta_q) tensor_scalar runs in the packed DVE mode
(measured 481ns vs 748ns with fp32 src).

Per-query work and engine assignment (measured ns):
    h1   = relu(gamma + beta_q)     DVE tensor_scalar            481
    p2   = w2^T h1                  PE, 2 matmuls N=512          2x216
    h2   = relu(p2)                 ACT 7/8 queries (1114), DVE 1/8 (1224)
                                    b2 == 0 by spec -> no bias operand
    p3   = w3p^T h2                 PE, 2 col-tiled matmuls      2x216
    stage <- p3                     DVE copy [128,512]/2q        681
With the 27:5 relu split and copies on DVE, ACT and DVE both carry
~975ns/query; PE carries ~864ns/query plus hidden LDWEIGHTS.

The emission order software-pipelines the PE against the drain engines:
per step s it issues w2(s+SHIFT) and w3(s-W3LAG), so relu2(s) has
multiple query-slots to finish before the PE consumes h2(s).
PSUM: 3 x p2 [128,1024] (2 banks each) + 2 x p3 [128,512] = 8 banks.

Output is staged and DMA'd as bf16 (halves DMA bytes); host upcasts.
Sharding: 8 cores x (batch, 256 queries). Weights replicated.
"""

import numpy as np

B, NQ, NK, H, HD = 2, 1024, 1024, 8, 128
NCORES = 8
CPB = NCORES // B          # cores per batch = 4
QPC = NQ // CPB            # queries per core = 256
KT = 512                   # matmul moving free dim (one PSUM bank fp32)
NGR = 8                    # 2-query groups per staging round
RQ = 2 * NGR               # queries per staging round = 16
W3P = 32                   # w3 padded to 32 output columns (col-group width)
# relu2 engine split: queries with q % 32 in this set run on DVE, rest ACT
DVE_RELU_SET = frozenset((4, 10, 17, 23, 29))
SHIFT = 2                  # w2 matmuls run this many queries ahead of relu2
W3LAG = 2                  # w3 matmuls trail relu2 by this many queries

_CACHE = {}


def _build_nc(with_bias):
    from contextlib import ExitStack

    import concourse.bass as bass
    import concourse.tile as tile
    from concourse import bacc, mybir
    from concourse.alu_op_type import AluOpType

    f32 = mybir.dt.float32
    bf16 = mybir.dt.bfloat16
    Relu = mybir.ActivationFunctionType.Relu

    nc = bacc.Bacc(
        "TRN2",
        target_bir_lowering=False,
        debug=False,
        enable_asserts=True,
        num_devices=NCORES,
    )

    gamma_d = nc.dram_tensor("gamma", (HD, NK), bf16, kind="ExternalInput").ap()
    beta_d = nc.dram_tensor("beta", (HD, QPC), f32, kind="ExternalInput").ap()
    w2_d = nc.dram_tensor("w2", (HD, HD), f32, kind="ExternalInput").ap()
    w3p_d = nc.dram_tensor("w3p", (HD, W3P), f32, kind="ExternalInput").ap()
    b2_d = nc.dram_tensor("b2", (HD, 1), f32, kind="ExternalInput").ap()
    out_d = nc.dram_tensor("out", (H, QPC, NK), bf16, kind="ExternalOutput").ap()

    with tile.TileContext(nc) as tc:
        with ExitStack() as ctx:
            consts = ctx.enter_context(tc.tile_pool(name="consts", bufs=1))
            h1p = ctx.enter_context(tc.tile_pool(name="h1p", bufs=6))
            h2p = ctx.enter_context(tc.tile_pool(name="h2p", bufs=6))
            stagep = ctx.enter_context(tc.tile_pool(name="stagep", bufs=2))
            ps2 = ctx.enter_context(tc.tile_pool(name="ps2", bufs=3, space="PSUM"))
            ps3 = ctx.enter_context(tc.tile_pool(name="ps3", bufs=2, space="PSUM"))

            # gamma feeds the first op of the pipeline: load it first
            gamma = consts.tile([HD, NK], bf16)
            nc.sync.dma_start(gamma, gamma_d)
            w2 = consts.tile([HD, HD], f32)
            nc.sync.dma_start(w2, w2_d)
            w3p = consts.tile([HD, W3P], f32)
            nc.sync.dma_start(w3p, w3p_d)
            b2 = consts.tile([HD, 1], f32)
            nc.sync.dma_start(b2, b2_d)
            beta = consts.tile([HD, QPC], f32)
            nc.sync.dma_start(beta, beta_d)

            w2r = consts.tile([HD, HD], bf16)
            nc.vector.tensor_copy(w2r, w2)
            w3pr = consts.tile([HD, W3P], bf16)
            nc.vector.tensor_copy(w3pr, w3p)

            h1t = [None] * QPC   # h1 tiles in flight
            h2t = [None] * QPC   # h2 tiles in flight
            p2t = [None] * QPC   # p2 psum tiles in flight
            p3t = {}             # group -> p3 psum tile
            staget = {}          # round -> stage tile

            def emit_h1(q):
                h1 = h1p.tile([HD, NK], bf16, tag="h1", name="h1")
                nc.vector.tensor_scalar(
                    h1, gamma, beta[:, q:q + 1], 0.0,
                    AluOpType.add, AluOpType.max,
                )
                h1t[q] = h1

            def emit_w2(q):
                p2 = ps2.tile([HD, NK], f32, tag="p2", name="p2")
                for kh in range(2):
                    nc.tensor.matmul(
                        p2[:, kh * KT:(kh + 1) * KT],
                        w2r,
                        h1t[q][:, kh * KT:(kh + 1) * KT],
                        start=True,
                        stop=True,
                    )
                p2t[q] = p2
                h1t[q] = None

            def emit_relu2(q):
                h2 = h2p.tile([HD, NK], bf16, tag="h2", name="h2")
                if with_bias:
                    nc.scalar.activation(h2, p2t[q], Relu, bias=b2)
                elif q % 32 in DVE_RELU_SET:
                    nc.vector.tensor_scalar(
                        h2, p2t[q], 0.0, None, AluOpType.max,
                    )
                else:
                    nc.scalar.activation(h2, p2t[q], Relu)
                h2t[q] = h2
                p2t[q] = None

            def emit_w3(q):
                g = q // 2
                if q % 2 == 0:
                    p3t[g] = ps3.tile([128, KT], f32, tag="p3", name="p3")
                for kh in range(2):
                    j = 2 * (q % 2) + kh
                    nc.tensor.matmul(
                        p3t[g][32 * j:32 * (j + 1), :],
                        w3pr,
                        h2t[q][:, kh * KT:(kh + 1) * KT],
                        start=True,
                        stop=True,
                        tile_position=(0, 32 * j),
                    )
                h2t[q] = None

            last_round = QPC // RQ - 1

            def emit_copy(g):
                # drain group g (queries 2g, 2g+1) into its round's stage.
                # In the last round ACT has run out of relu work, so split
                # the drains between ACT and DVE to shorten the tail.
                r, gi = divmod(g, NGR)
                if gi == 0:
                    staget[r] = stagep.tile([128, NGR * KT], bf16, tag="stage", name="stage")
                dst = staget[r][:, gi * KT:(gi + 1) * KT]
                if r == last_round and gi % 2 == 0:
                    nc.scalar.copy(dst, p3t[g])
                else:
                    nc.vector.tensor_copy(dst, p3t[g])
                del p3t[g]

            def emit_dma(r, c0, cn, final):
                # Staging layout: slot gi holds the 4-tile group for queries
                # (q0+2gi, q0+2gi+1); partition block 32j+0..8 holds heads for
                # (q offset j//2, k half j%2).  j 0/1 go out on the sync HWDGE
                # queue, j 2/3 on the gpsimd SWDGE queue (parallel channel,
                # gpsimd is otherwise idle).
                q0 = r * RQ
                for j in range(4):
                    dest = bass.AP(
                        tensor=out_d.tensor,
                        offset=out_d.offset + (q0 + 2 * c0 + (j // 2)) * NK
                        + (j % 2) * KT,
                        ap=[[QPC * NK, H], [2 * NK, cn], [1, KT]],
                    )
                    nc.sync.dma_start(
                        dest,
                        staget[r][32 * j:32 * j + H, c0 * KT:(c0 + cn) * KT],
                    )
                if final:
                    del staget[r]

            def after_copy(g):
                # fire DMAs once their slots are staged; the last round goes
                # out in two half-round chunks so the tail transfer is small
                r, gi = divmod(g, NGR)
                if gi == NGR - 1:
                    if r == last_round:
                        emit_dma(r, NGR // 2, NGR // 2, final=True)
                    else:
                        emit_dma(r, 0, NGR, final=True)
                elif r == last_round and gi == NGR // 2 - 1:
                    emit_dma(r, 0, NGR // 2, final=False)

            # --- software-pipelined emission ---
            for q in range(min(SHIFT + 1, QPC)):
                emit_h1(q)
            for q in range(min(SHIFT, QPC)):
                emit_w2(q)

            for s in range(QPC + W3LAG + 1):
                if s < QPC:
                    emit_relu2(s)
                if s + SHIFT + 1 < QPC:
                    emit_h1(s + SHIFT + 1)
                if s + SHIFT < QPC:
                    emit_w2(s + SHIFT)
                w3q = s - W3LAG
                if 0 <= w3q < QPC:
                    emit_w3(w3q)
                # drain a group one step after its second query's w3
                cpq = w3q - 1
                if cpq >= 1 and cpq % 2 == 1 and (cpq // 2) in p3t:
                    g = cpq // 2
                    emit_copy(g)
                    after_copy(g)
            # flush remaining groups
            for g in sorted(p3t.keys()):
                emit_copy(g)
                after_copy(g)

    nc.compile()
    return nc


def _get_nc(with_bias=False):
    key = ("nc", bool(with_bias))
    if key not in _CACHE:
        _CACHE[key] = _build_nc(with_bias)
    return _CACHE[key]


def make_in_maps(query_coords, key_coords, w1, b1, w2, b2, w3):
    """Host-side shard prep: per-core gamma/beta + replicated weights."""
    qc = np.asarray(query_coords, np.float32)
    kc = np.asarray(key_coords, np.float32)
    w1 = np.asarray(w1, np.float32)
    b1 = np.asarray(b1, np.float32)
    w2 = np.asarray(w2, np.float32)
    b2 = np.asarray(b2, np.float32)
    w3 = np.asarray(w3, np.float32)

    w3p = np.zeros((HD, W3P), np.float32)
    w3p[:, :H] = w3
    b2c = np.ascontiguousarray(b2.reshape(HD, 1))
    w2c = np.ascontiguousarray(w2)

    import ml_dtypes

    in_maps = []
    for c in range(NCORES):
        b = c // CPB
        q0 = (c % CPB) * QPC
        gamma = np.ascontiguousarray(
            (-(kc[b] @ w1).T).astype(ml_dtypes.bfloat16)         # (128, NK)
        )
        beta = np.ascontiguousarray(
            (qc[b, q0:q0 + QPC] @ w1).T + b1[:, None]            # (128, QPC)
        )
        in_maps.append(
            {"gamma": gamma, "beta": beta, "w2": w2c, "w3p": w3p, "b2": b2c}
        )
    return in_maps


def assemble_output(results, b3):
    """Gather per-core [H, QPC, NK] bf16 results into (B, H, NQ, NK) f32."""
    b3 = np.asarray(b3, np.float32)
    out = np.empty((B, H, NQ, NK), np.float32)
    for c in range(NCORES):
        b = c // CPB
        q0 = (c % CPB) * QPC
        out[b, :, q0:q0 + QPC, :] = np.asarray(results[c]["out"],
                                               dtype=np.float32)
    if np.any(b3):
        out += b3.reshape(1, H, 1, 1)
    return out


def kernel(**inputs):
    from concourse.bass_utils import run_bass_kernel_spmd

    in_maps = make_in_maps(
        inputs["query_coords"],
        inputs["key_coords"],
        inputs["w1"],
        inputs["b1"],
        inputs["w2"],
        inputs["b2"],
        inputs["w3"],
    )
    with_bias = bool(np.any(np.asarray(inputs["b2"], np.float32)))
    nc = _get_nc(with_bias)
    res = run_bass_kernel_spmd(nc, in_maps, list(range(NCORES)))
    return assemble_output(res.results, inputs["b3"])


# revision 6
# speedup vs baseline: 1.0226x; 1.0027x over previous
"""Continuous positional bias kernel for Trainium2 (8 NeuronCores), v2.

Reference computation (per batch b):
    rel[q,k,:] = query_coords[b,q,:] - key_coords[b,k,:]
    h1 = relu(rel @ w1 + b1)      # (Nq,Nk,128)
    h2 = relu(h1 @ w2 + b2)       # (Nq,Nk,128)
    out[b,:,q,k] = (h2 @ w3 + b3).T

Layer 1 is linear in rel = q - k:
    w1^T rel + b1 = (w1^T q + b1) + (-w1^T k) = beta[:,q] + gamma[:,k]
gamma/beta computed on host; gamma held in SBUF as bf16 so the per-query
h1 = relu(gamma + beta_q) tensor_scalar runs in the packed DVE mode
(measured 481ns vs 748ns with fp32 src).

Per-query work and engine assignment (measured ns):
    h1   = relu(gamma + beta_q)     DVE tensor_scalar            481
    p2   = w2^T h1                  PE, 2 matmuls N=512          2x216
    h2   = relu(p2)                 ACT 7/8 queries (1114), DVE 1/8 (1224)
                                    b2 == 0 by spec -> no bias operand
    p3   = w3p^T h2                 PE, 2 col-tiled matmuls      2x216
    stage <- p3                     DVE copy [128,512]/2q        681
With the 27:5 relu split and copies on DVE, ACT and DVE both carry
~975ns/query; PE carries ~864ns/query plus hidden LDWEIGHTS.

The emission order software-pipelines the PE against the drain engines:
per step s it issues w2(s+SHIFT) and w3(s-W3LAG), so relu2(s) has
multiple query-slots to finish before the PE consumes h2(s).
PSUM: 3 x p2 [128,1024] (2 banks each) + 2 x p3 [128,512] = 8 banks.

Output is staged and DMA'd as bf16 (halves DMA bytes); host upcasts.
Sharding: 8 cores x (batch, 256 queries). Weights replicated.
"""

import numpy as np

B, NQ, NK, H, HD = 2, 1024, 1024, 8, 128
NCORES = 8
CPB = NCORES // B          # cores per batch = 4
QPC = NQ // CPB            # queries per core = 256
KT = 512                   # matmul moving free dim (one PSUM bank fp32)
NGR = 8                    # 2-query groups per staging round
RQ = 2 * NGR               # queries per staging round = 16
W3P = 32                   # w3 padded to 32 output columns (col-group width)
# relu2 engine split: queries with q % 32 in this set run on DVE, rest ACT
DVE_RELU_SET = frozenset((4, 10, 17, 23, 29))
SHIFT = 2                  # w2 matmuls run this many queries ahead of relu2
W3LAG = 2                  # w3 matmuls trail relu2 by this many queries

_CACHE = {}


def _build_nc(with_bias):
    from contextlib import ExitStack

    import concourse.bass as bass
    import concourse.tile as tile
    from concourse import bacc, mybir
    from concourse.alu_op_type import AluOpType

    f32 = mybir.dt.float32
    bf16 = mybir.dt.bfloat16
    Relu = mybir.ActivationFunctionType.Relu

    nc = bacc.Bacc(
        "TRN2",
        target_bir_lowering=False,
        debug=False,
        enable_asserts=True,
        num_devices=NCORES,
    )

    gamma_d = nc.dram_tensor("gamma", (HD, NK), bf16, kind="ExternalInput").ap()
    beta_d = nc.dram_tensor("beta", (HD, QPC), f32, kind="ExternalInput").ap()
    w2_d = nc.dram_tensor("w2", (HD, HD), f32, kind="ExternalInput").ap()
    w3p_d = nc.dram_tensor("w3p", (HD, W3P), f32, kind="ExternalInput").ap()
    b2_d = nc.dram_tensor("b2", (HD, 1), f32, kind="ExternalInput").ap()
    out_d = nc.dram_tensor("out", (H, QPC, NK), bf16, kind="ExternalOutput").ap()

    with tile.TileContext(nc) as tc:
        with ExitStack() as ctx:
            consts = ctx.enter_context(tc.tile_pool(name="consts", bufs=1))
            h1p = ctx.enter_context(tc.tile_pool(name="h1p", bufs=6))
            h2p = ctx.enter_context(tc.tile_pool(name="h2p", bufs=6))
            stagep = ctx.enter_context(tc.tile_pool(name="stagep", bufs=2))
            ps2 = ctx.enter_context(tc.tile_pool(name="ps2", bufs=3, space="PSUM"))
            ps3 = ctx.enter_context(tc.tile_pool(name="ps3", bufs=2, space="PSUM"))

            # Input loads gate pipeline start-up: split them across the two
            # HWDGE queues (sync + scalar) so the first h1 can issue ~3us
            # earlier.  beta is split so the first rounds' columns arrive
            # with the first gamma half.
            BQ0 = 64
            beta_a = consts.tile([HD, BQ0], f32)
            nc.sync.dma_start(
                beta_a,
                bass.AP(tensor=beta_d.tensor, offset=beta_d.offset,
                        ap=[[QPC, HD], [1, BQ0]]),
            )
            gamma = consts.tile([HD, NK], bf16)
            nc.sync.dma_start(
                gamma[:, 0:NK // 2],
                bass.AP(tensor=gamma_d.tensor, offset=gamma_d.offset,
                        ap=[[NK, HD], [1, NK // 2]]),
            )
            nc.scalar.dma_start(
                gamma[:, NK // 2:NK],
                bass.AP(tensor=gamma_d.tensor,
                        offset=gamma_d.offset + NK // 2,
                        ap=[[NK, HD], [1, NK // 2]]),
            )
            w2 = consts.tile([HD, HD], f32)
            nc.sync.dma_start(w2, w2_d)
            w3p = consts.tile([HD, W3P], f32)
            nc.sync.dma_start(w3p, w3p_d)
            b2 = consts.tile([HD, 1], f32)
            nc.sync.dma_start(b2, b2_d)
            beta_b = consts.tile([HD, QPC - BQ0], f32)
            nc.scalar.dma_start(
                beta_b,
                bass.AP(tensor=beta_d.tensor, offset=beta_d.offset + BQ0,
                        ap=[[QPC, HD], [1, QPC - BQ0]]),
            )

            w2r = consts.tile([HD, HD], bf16)
            nc.vector.tensor_copy(w2r, w2)
            w3pr = consts.tile([HD, W3P], bf16)
            nc.vector.tensor_copy(w3pr, w3p)

            h1t = [None] * QPC   # h1 tiles in flight
            h2t = [None] * QPC   # h2 tiles in flight
            p2t = [None] * QPC   # p2 psum tiles in flight
            p3t = {}             # group -> p3 psum tile
            staget = {}          # round -> stage tile

            def emit_h1(q):
                h1 = h1p.tile([HD, NK], bf16, tag="h1", name="h1")
                bcol = (beta_a[:, q:q + 1] if q < BQ0
                        else beta_b[:, q - BQ0:q - BQ0 + 1])
                nc.vector.tensor_scalar(
                    h1, gamma, bcol, 0.0,
                    AluOpType.add, AluOpType.max,
                )
                h1t[q] = h1

            def emit_w2(q):
                p2 = ps2.tile([HD, NK], f32, tag="p2", name="p2")
                for kh in range(2):
                    nc.tensor.matmul(
                        p2[:, kh * KT:(kh + 1) * KT],
                        w2r,
                        h1t[q][:, kh * KT:(kh + 1) * KT],
                        start=True,
                        stop=True,
                    )
                p2t[q] = p2
                h1t[q] = None

            def emit_relu2(q):
                h2 = h2p.tile([HD, NK], bf16, tag="h2", name="h2")
                if with_bias:
                    nc.scalar.activation(h2, p2t[q], Relu, bias=b2)
                elif q % 32 in DVE_RELU_SET:
                    nc.vector.tensor_scalar(
                        h2, p2t[q], 0.0, None, AluOpType.max,
                    )
                else:
                    nc.scalar.activation(h2, p2t[q], Relu)
                h2t[q] = h2
                p2t[q] = None

            def emit_w3(q):
                g = q // 2
                if q % 2 == 0:
                    p3t[g] = ps3.tile([128, KT], f32, tag="p3", name="p3")
                for kh in range(2):
                    j = 2 * (q % 2) + kh
                    nc.tensor.matmul(
                        p3t[g][32 * j:32 * (j + 1), :],
                        w3pr,
                        h2t[q][:, kh * KT:(kh + 1) * KT],
                        start=True,
                        stop=True,
                        tile_position=(0, 32 * j),
                    )
                h2t[q] = None

            last_round = QPC // RQ - 1

            def emit_copy(g):
                # drain group g (queries 2g, 2g+1) into its round's stage.
                # In the last round ACT has run out of relu work, so split
                # the drains between ACT and DVE to shorten the tail.
                r, gi = divmod(g, NGR)
                if gi == 0:
                    staget[r] = stagep.tile([128, NGR * KT], bf16, tag="stage", name="stage")
                dst = staget[r][:, gi * KT:(gi + 1) * KT]
                if r == last_round and gi % 2 == 0:
                    nc.scalar.copy(dst, p3t[g])
                else:
                    nc.vector.tensor_copy(dst, p3t[g])
                del p3t[g]

            def emit_dma(r, c0, cn, final):
                # Staging layout: slot gi holds the 4-tile group for queries
                # (q0+2gi, q0+2gi+1); partition block 32j+0..8 holds heads for
                # (q offset j//2, k half j%2).  j 0/1 go out on the sync HWDGE
                # queue, j 2/3 on the gpsimd SWDGE queue (parallel channel,
                # gpsimd is otherwise idle).
                q0 = r * RQ
                for j in range(4):
                    dest = bass.AP(
                        tensor=out_d.tensor,
                        offset=out_d.offset + (q0 + 2 * c0 + (j // 2)) * NK
                        + (j % 2) * KT,
                        ap=[[QPC * NK, H], [2 * NK, cn], [1, KT]],
                    )
                    nc.sync.dma_start(
                        dest,
                        staget[r][32 * j:32 * j + H, c0 * KT:(c0 + cn) * KT],
                    )
                if final:
                    del staget[r]

            def after_copy(g):
                # fire DMAs once their slots are staged; the last round goes
                # out in two half-round chunks so the tail transfer is small
                r, gi = divmod(g, NGR)
                if gi == NGR - 1:
                    if r == last_round:
                        emit_dma(r, NGR // 2, NGR // 2, final=True)
                    else:
                        emit_dma(r, 0, NGR, final=True)
                elif r == last_round and gi == NGR // 2 - 1:
                    emit_dma(r, 0, NGR // 2, final=False)

            # --- software-pipelined emission ---
            for q in range(min(SHIFT + 1, QPC)):
                emit_h1(q)
            for q in range(min(SHIFT, QPC)):
                emit_w2(q)

            for s in range(QPC + W3LAG + 1):
                if s < QPC:
                    emit_relu2(s)
                if s + SHIFT + 1 < QPC:
                    emit_h1(s + SHIFT + 1)
                if s + SHIFT < QPC:
                    emit_w2(s + SHIFT)
                w3q = s - W3LAG
                if 0 <= w3q < QPC:
                    emit_w3(w3q)
                # drain a group one step after its second query's w3
                cpq = w3q - 1
                if cpq >= 1 and cpq % 2 == 1 and (cpq // 2) in p3t:
                    g = cpq // 2
                    emit_copy(g)
                    after_copy(g)
            # flush remaining groups
            for g in sorted(p3t.keys()):
                emit_copy(g)
                after_copy(g)

    nc.compile()
    return nc


def _get_nc(with_bias=False):
    key = ("nc", bool(with_bias))
    if key not in _CACHE:
        _CACHE[key] = _build_nc(with_bias)
    return _CACHE[key]


def make_in_maps(query_coords, key_coords, w1, b1, w2, b2, w3):
    """Host-side shard prep: per-core gamma/beta + replicated weights."""
    qc = np.asarray(query_coords, np.float32)
    kc = np.asarray(key_coords, np.float32)
    w1 = np.asarray(w1, np.float32)
    b1 = np.asarray(b1, np.float32)
    w2 = np.asarray(w2, np.float32)
    b2 = np.asarray(b2, np.float32)
    w3 = np.asarray(w3, np.float32)

    w3p = np.zeros((HD, W3P), np.float32)
    w3p[:, :H] = w3
    b2c = np.ascontiguousarray(b2.reshape(HD, 1))
    w2c = np.ascontiguousarray(w2)

    import ml_dtypes

    in_maps = []
    for c in range(NCORES):
        b = c // CPB
        q0 = (c % CPB) * QPC
        gamma = np.ascontiguousarray(
            (-(kc[b] @ w1).T).astype(ml_dtypes.bfloat16)         # (128, NK)
        )
        beta = np.ascontiguousarray(
            (qc[b, q0:q0 + QPC] @ w1).T + b1[:, None]            # (128, QPC)
        )
        in_maps.append(
            {"gamma": gamma, "beta": beta, "w2": w2c, "w3p": w3p, "b2": b2c}
        )
    return in_maps


def assemble_output(results, b3):
    """Gather per-core [H, QPC, NK] bf16 results into (B, H, NQ, NK) f32."""
    b3 = np.asarray(b3, np.float32)
    out = np.empty((B, H, NQ, NK), np.float32)
    for c in range(NCORES):
        b = c // CPB
        q0 = (c % CPB) * QPC
        out[b, :, q0:q0 + QPC, :] = np.asarray(results[c]["out"],
                                               dtype=np.float32)
    if np.any(b3):
        out += b3.reshape(1, H, 1, 1)
    return out


def kernel(**inputs):
    from concourse.bass_utils import run_bass_kernel_spmd

    in_maps = make_in_maps(
        inputs["query_coords"],
        inputs["key_coords"],
        inputs["w1"],
        inputs["b1"],
        inputs["w2"],
        inputs["b2"],
        inputs["w3"],
    )
    with_bias = bool(np.any(np.asarray(inputs["b2"], np.float32)))
    nc = _get_nc(with_bias)
    res = run_bass_kernel_spmd(nc, in_maps, list(range(NCORES)))
    return assemble_output(res.results, inputs["b3"])
